# revision 5
# baseline (speedup 1.0000x reference)
"""Trainium2 Bass kernel for a 5-layer bidirectional GRU (T=256, B=128, I=128, H=512, O=1).

Strategy:
  - Data-parallel over batch: 8 cores x 16 batch elements (SPMD, no collectives).
  - Everything feature-major [feature, token] so no transposes are ever needed:
      * recurrent matmul is weight-stationary: gh[3H, B] = sum_k WhhT[k].T @ h[k]
      * input-side gates gx[3H, tok] precomputed per 32-step block with big
        token-parallel matmuls (weights stationary across 512 token columns)
  - bf16 matmuls (fp32 is 4x slower on the PE), fp32 PSUM accumulate, fp32
    master hidden state, bf16 activations.
  - Both directions interleaved per step: two independent dependency chains
    keep the PE busy while the other direction's elementwise runs.
  - Activations ping-pong through internal DRAM between layers.
"""

import sys

sys.path.insert(0, "/opt/trn_rl_repo")

import numpy as np
import ml_dtypes

import concourse.bass as bass
import concourse.bacc as bacc
import concourse.mybir as mybir
import concourse.tile as tile
from concourse.vector_clock import ScopedClock, VectorClock
from concourse.bass_utils import run_bass_kernel_spmd

BF16 = mybir.dt.bfloat16
F32 = mybir.dt.float32
AF = mybir.ActivationFunctionType
OP = mybir.AluOpType

T, B, I, H, O, L = 256, 128, 128, 512, 1, 5
G3 = 3 * H  # 1536
NCORES = 8
BC = B // NCORES            # 16 batch per core
TOK = T * BC                # 4096 token columns per core
NBLK = T // 32              # 8 blocks of 32 timesteps
KH = H // 128               # 4 k-chunks of the hidden dim
M3 = G3 // 128              # 12 m-chunks of the gate dim
P = 128


class ChunkedDrainTC(tile.TileContext):
    """Work around walrus's 2-sync-wait limit on the kernel-tail drain by
    splitting the final drain into several drains with <=2 waits each."""

    def _drain_and_barrier(self, tick_clock, wait_clock):
        gc = tick_clock.global_clock
        n = len(gc)
        for i0 in range(0, n, 2):
            vec = [0] * n
            any_set = False
            for i in range(i0, min(i0 + 2, n)):
                vec[i] = gc[i]
                any_set = any_set or gc[i] > 0
            if not any_set:
                continue
            di = self.nc.sync.drain()
            wait_clock.add_sem_waits(di.ins, ScopedClock({None: VectorClock(vec)}))
        self.nc.all_engine_barrier()
        popped = self.nc._tile_sem_poison_stack.pop()
        assert popped is self._sem_poison
        self.nc.clear_and_free_semaphores(list(self.sems.allocated().values()))
        self.nc.all_engine_barrier()


def build_bass(t_steps=T, n_layers=L, repeat=1, elem_mode="full", whh_fp8=True):
    """Build the SPMD per-core program. Returns nc.

    repeat > 1 re-runs the whole network that many times (for timing by
    differencing out the fixed dispatch overhead)."""
    nblk = t_steps // 32
    tok = t_steps * BC
    WHH_DT = mybir.dt.float8e4 if whh_fp8 else BF16

    nc = bacc.Bacc(None)

    # ---- external I/O ----
    x_in = nc.dram_tensor("x", [1, P, tok], BF16, kind="ExternalInput")
    wih0_in = nc.dram_tensor("wih0", [2, I, G3], BF16, kind="ExternalInput")
    whh_in = nc.dram_tensor("whh", [n_layers, 2, H, G3], WHH_DT, kind="ExternalInput")
    gxb_in = nc.dram_tensor("gxb", [n_layers, 2, P, M3], F32, kind="ExternalInput")
    bhnb_in = nc.dram_tensor("bhnb", [n_layers, 2, 1, H], BF16, kind="ExternalInput")
    out_d = nc.dram_tensor("out", [1, tok], F32, kind="ExternalOutput")
    if n_layers > 1:
        wih_in = nc.dram_tensor(
            "wih", [n_layers - 1, 2, 2 * H, G3], BF16, kind="ExternalInput"
        )
    fcw_in = nc.dram_tensor("fcw", [2 * H, 1], BF16, kind="ExternalInput")
    fcb_in = nc.dram_tensor("fcb", [1, 1], F32, kind="ExternalInput")

    # ---- internal DRAM activation ping-pong [k-chunk, 128, tok] ----
    act_a = nc.dram_tensor("act_a", [2 * KH, P, tok], BF16)
    act_b = nc.dram_tensor("act_b", [2 * KH, P, tok], BF16)

    with ChunkedDrainTC(nc) as tc:
        with (
            tc.tile_pool(name="wpool", bufs=1) as wpool,        # weights
            tc.tile_pool(name="state", bufs=1) as state,        # per-layer state
            tc.tile_pool(name="stage", bufs=3) as stage_pool,   # act staging
            tc.tile_pool(name="tmp", bufs=4) as tmp_pool,       # elementwise temps
            tc.tile_pool(name="ghps", bufs=4, space="PSUM") as ghps_pool,
            tc.tile_pool(name="gxps", bufs=3, space="PSUM") as gxps_pool,
        ):
            for layer in [ly for _ in range(repeat) for ly in range(n_layers)]:
                ki = 1 if layer == 0 else 2 * KH  # input k-chunks
                act_in = x_in if layer == 0 else (act_a if layer % 2 == 1 else act_b)
                act_out = act_a if layer % 2 == 0 else act_b

                # ---- load weights/biases for both dirs ----
                whh_sb, wih_sb, gxb_sb, bhn_sb = [], [], [], []
                for d in range(2):
                    w = wpool.tile([P, KH, G3], WHH_DT, tag=f"whh{d}")
                    nc.sync.dma_start(
                        w[:], whh_in[layer, d].rearrange("(ko p) m -> p ko m", p=P)
                    )
                    whh_sb.append(w)
                    wi = wpool.tile([P, ki, G3], BF16, tag=f"wih{d}")
                    src = (
                        wih0_in[d]
                        if layer == 0
                        else wih_in[layer - 1, d]
                    ).rearrange("(ko p) m -> p ko m", p=P)
                    nc.sync.dma_start(wi[:], src)
                    wih_sb.append(wi)
                    gb = wpool.tile([P, M3], F32, tag=f"gxb{d}")
                    nc.sync.dma_start(gb[:], gxb_in[layer, d])
                    gxb_sb.append(gb)
                    bh = wpool.tile([1, H], BF16, tag=f"bhnb{d}")
                    nc.sync.dma_start(bh[:], bhnb_in[layer, d])
                    bhn_sb.append(bh)
                ones_sb = wpool.tile([1, BC], BF16, tag="ones")
                nc.vector.memset(ones_sb[:], 1.0)

                # ---- per-layer state ----
                h_hist = []     # bf16 hidden history ring [128, KH, 2, 32*BC]
                gx_ring = []    # bf16 input-gate ring  [128, M3, 2, 32*BC]
                for d in range(2):
                    hh = state.tile([P, KH, 2, 32 * BC], BF16, tag=f"hh{d}")
                    nc.vector.memset(hh[:], 0.0)
                    h_hist.append(hh)
                    gxr = state.tile([P, M3, 2, 32 * BC], BF16, tag=f"gx{d}", name=f"gx{d}")
                    gx_ring.append(gxr)

                stage_sb = [None, None]

                def emit_gx_stage(d, tb):
                    """DMA the act tokens of t-block tb into SBUF staging."""
                    st = stage_pool.tile([P, ki, 32 * BC], BF16, tag="stage", name="st")
                    nc.sync.dma_start(
                        st[:],
                        act_in[0:ki, :, tb * 32 * BC : (tb + 1) * 32 * BC].rearrange(
                            "k p c -> p k c"
                        ),
                    )
                    stage_sb[d] = st

                def emit_gx_group(d, tb, m):
                    """Input-side gate matmuls for one m-chunk of t-block tb."""
                    par = tb % 2
                    st = stage_sb[d]
                    ps = gxps_pool.tile([P, 32 * BC], F32, tag="gxps", name="gxps")
                    for k in range(ki):
                        nc.tensor.matmul(
                            ps[:],
                            wih_sb[d][:, k, m * P : (m + 1) * P],
                            st[:, k, :],
                            start=(k == 0),
                            stop=(k == ki - 1),
                        )
                    # copy psum -> ring with per-feature bias, f32 -> bf16
                    nc.scalar.activation(
                        gx_ring[d][:, m, par, :],
                        ps[:],
                        AF.Identity,
                        bias=gxb_sb[d][:, m : m + 1],
                    )

                def emit_flush(d, tb):
                    """Store finished hidden states of t-block tb to DRAM act."""
                    par = tb % 2
                    nc.sync.dma_start(
                        act_out[
                            d * KH : (d + 1) * KH, :, tb * 32 * BC : (tb + 1) * 32 * BC
                        ].rearrange("k p c -> p k c"),
                        h_hist[d][:, :, par, :],
                    )

                def emit_step(d, t, ghp):
                    """Recurrent matmuls for dir d into its half of the unified
                    PSUM tile ghp [128, 2, M3, BC]."""
                    td = t if d == 0 else (t_steps - 1 - t)  # token this step computes
                    prev = td - 1 if d == 0 else td + 1      # token holding h_{prev}
                    slp, pap = prev % 32, (prev // 32) % 2

                    base = d * M3 * BC
                    rhs = h_hist[d][:, :, pap, slp * BC : (slp + 1) * BC]
                    for m in range(M3):
                        has_bias_mm = m >= 8  # n chunks get b_hn via K=1 matmul
                        sl_ = slice(base + m * BC, base + (m + 1) * BC)
                        for k in range(KH):
                            nc.tensor.matmul(
                                ghp[:, sl_],
                                whh_sb[d][:, k, m * P : (m + 1) * P],
                                rhs[:, k, :],
                                start=(k == 0),
                                stop=(k == KH - 1) and not has_bias_mm,
                            )
                        if has_bias_mm:
                            nc.tensor.matmul(
                                ghp[:, sl_],
                                bhn_sb[d][0:1, (m - 8) * P : (m - 7) * P],
                                ones_sb[0:1, :],
                                start=False,
                                stop=True,
                            )

                def emit_elem(d, t, ghp):
                    """Gate nonlinearity + state update for one direction:
                    bf16 SBUF temps, 6 DVE + 2 ACT ops, short serial chain so
                    the other direction's matmuls overlap on the PE."""
                    ghv = ghp[:].rearrange("p (d m b) -> p d m b", d=2, b=BC)
                    td = t if d == 0 else (t_steps - 1 - t)
                    sl, pa = td % 32, (td // 32) % 2
                    prev = td - 1 if d == 0 else td + 1
                    slp, pap = prev % 32, (prev // 32) % 2
                    h_new = h_hist[d][:, :, pa, sl * BC : (sl + 1) * BC]
                    if elem_mode == "dummy":
                        nc.vector.tensor_copy(h_new, ghv[:, d, 0:KH, :])
                        return
                    gx_rz = gx_ring[d][:, 0:8, pa, sl * BC : (sl + 1) * BC]
                    gx_n = gx_ring[d][:, 8:12, pa, sl * BC : (sl + 1) * BC]
                    h_prev = h_hist[d][:, :, pap, slp * BC : (slp + 1) * BC]

                    trz = tmp_pool.tile([P, 8, BC], BF16, tag=f"trz{d}", name="trz")
                    nc.vector.tensor_add(trz[:], ghv[:, d, 0:8, :], gx_rz)
                    rz = tmp_pool.tile([P, 8, BC], BF16, tag=f"rz{d}", name="rz")
                    nc.scalar.activation(rz[:], trz[:], AF.Sigmoid)
                    tn = tmp_pool.tile([P, KH, BC], BF16, tag=f"tn{d}", name="tn")
                    nc.vector.tensor_mul(tn[:], ghv[:, d, 8:12, :], rz[:, 0:KH, :])
                    tn2 = tmp_pool.tile([P, KH, BC], BF16, tag=f"tn2{d}", name="tn2")
                    nc.vector.tensor_add(tn2[:], tn[:], gx_n)
                    nt = tmp_pool.tile([P, KH, BC], BF16, tag=f"nt{d}", name="nt")
                    nc.scalar.activation(nt[:], tn2[:], AF.Tanh)
                    dt_ = tmp_pool.tile([P, KH, BC], BF16, tag=f"dt{d}", name="dt")
                    nc.vector.tensor_sub(dt_[:], h_prev, nt[:])
                    dt2 = tmp_pool.tile([P, KH, BC], BF16, tag=f"dt2{d}", name="dt2")
                    nc.vector.tensor_mul(dt2[:], dt_[:], rz[:, KH : 2 * KH, :])
                    nc.vector.tensor_add(h_new, nt[:], dt2[:])

                # gx schedule: 24 m-groups (both dirs) spread over a block's 32
                # steps; group g of the NEXT block is emitted during step
                # ts where cumulative quota passes g.
                def gx_groups_due(ts):
                    lo = (ts * 24) // 32
                    hi = ((ts + 1) * 24) // 32
                    return range(lo, hi)

                # ---- prologue: stage + gx for the first consumed blocks ----
                emit_gx_stage(0, 0)
                emit_gx_stage(1, nblk - 1)
                for m in range(M3):
                    emit_gx_group(0, 0, m)
                    emit_gx_group(1, nblk - 1, m)
                for tb in range(nblk):
                    if tb < nblk - 1:
                        emit_gx_stage(0, tb + 1)
                        emit_gx_stage(1, nblk - 2 - tb)
                    for ts in range(32):
                        t = tb * 32 + ts
                        ghp = ghps_pool.tile(
                            [P, 2 * M3 * BC], F32, tag="ghps", name="ghps"
                        )
                        emit_step(0, t, ghp)
                        emit_step(1, t, ghp)
                        if tb < nblk - 1:
                            for g in gx_groups_due(ts):
                                d, m = g % 2, g // 2
                                emit_gx_group(d, tb + 1 if d == 0 else nblk - 2 - tb, m)
                        emit_elem(0, t, ghp)
                        emit_elem(1, t, ghp)
                    emit_flush(0, tb)
                    emit_flush(1, nblk - 1 - tb)

            # ---- final FC + sigmoid over the last layer's output ----
            act_fin = act_a if (n_layers - 1) % 2 == 0 else act_b
            fcw_sb = wpool.tile([P, 2 * KH, 1], BF16, tag="fcw")
            nc.sync.dma_start(fcw_sb[:], fcw_in.rearrange("(ko p) n -> p ko n", p=P))
            fcb_sb = wpool.tile([1, 1], F32, tag="fcb")
            nc.sync.dma_start(fcb_sb[:], fcb_in[:])
            out_sb = state.tile([1, tok], F32, tag="osb")
            for blk in range(nblk):
                st = stage_pool.tile([P, 2 * KH, 32 * BC], BF16, tag="stage")
                nc.sync.dma_start(
                    st[:],
                    act_fin[:, :, blk * 32 * BC : (blk + 1) * 32 * BC].rearrange(
                        "k p c -> p k c"
                    ),
                )
                ps = gxps_pool.tile([1, 32 * BC], F32, tag="gxps")
                for k in range(2 * KH):
                    nc.tensor.matmul(
                        ps[:],
                        fcw_sb[:, k, :],
                        st[:, k, :],
                        start=(k == 0),
                        stop=(k == 2 * KH - 1),
                    )
                nc.scalar.activation(
                    out_sb[:, blk * 32 * BC : (blk + 1) * 32 * BC],
                    ps[:],
                    AF.Sigmoid,
                    bias=fcb_sb[:, 0:1],
                )
            nc.sync.dma_start(out_d[:], out_sb[:])

    nc.finalize()
    return nc


def prep_inputs(input_seq, W_ih0, W_hh0, b_ih0, b_hh0, W_ih, W_hh, b_ih, b_hh,
                fc_w, fc_b, t_steps=T, n_layers=L, whh_fp8=True):
    """Host-side prep: transposes, bias folding, bf16 casts. Returns in_maps."""
    bf = ml_dtypes.bfloat16
    whh_dt = ml_dtypes.float8_e4m3fn if whh_fp8 else bf
    tok = t_steps * BC

    wih0 = np.ascontiguousarray(np.transpose(np.asarray(W_ih0), (0, 2, 1))).astype(bf)
    whh_all = np.concatenate(
        [np.asarray(W_hh0)[None], np.asarray(W_hh)], axis=0
    )[:n_layers]
    whh = np.ascontiguousarray(np.transpose(whh_all, (0, 1, 3, 2))).astype(whh_dt)
    bih_all = np.concatenate([np.asarray(b_ih0)[None], np.asarray(b_ih)], axis=0)[:n_layers]
    bhh_all = np.concatenate([np.asarray(b_hh0)[None], np.asarray(b_hh)], axis=0)[:n_layers]

    # gx bias: b_ih everywhere + b_hh on the r,z gates only (b_hn rides separately)
    gxb = bih_all.copy()
    gxb[:, :, : 2 * H] += bhh_all[:, :, : 2 * H]
    gxb = np.ascontiguousarray(
        np.transpose(gxb.reshape(n_layers, 2, M3, P), (0, 1, 3, 2))
    ).astype(np.float32)
    bhnb = np.ascontiguousarray(bhh_all[:, :, None, 2 * H :]).astype(bf)

    base = {
        "wih0": wih0,
        "whh": whh,
        "gxb": gxb,
        "bhnb": bhnb,
        "fcw": np.ascontiguousarray(np.asarray(fc_w).T).astype(bf),
        "fcb": np.asarray(fc_b, dtype=np.float32).reshape(1, 1),
    }
    if n_layers > 1:
        base["wih"] = np.ascontiguousarray(
            np.transpose(np.asarray(W_ih), (0, 1, 3, 2))
        )[: n_layers - 1].astype(bf)

    x = np.asarray(input_seq)[:t_steps]
    in_maps = []
    for c in range(NCORES):
        xc = x[:, c * BC : (c + 1) * BC, :].reshape(tok, I).T  # [128, tok]
        m = dict(base)
        m["x"] = np.ascontiguousarray(xc)[None].astype(bf)
        in_maps.append(m)
    return in_maps


def assemble_output(results, t_steps=T):
    """results: list of per-core dicts with 'out' [1, tok] -> [T, B, 1] f32."""
    outs = []
    for c in range(NCORES):
        o = np.asarray(results[c]["out"]).reshape(t_steps, BC)
        outs.append(o)
    return np.stack(outs, axis=1).reshape(t_steps, B)[:, :, None].astype(np.float32)


def kernel(**inputs):
    nc = build_bass()
    in_maps = prep_inputs(**inputs)
    res = run_bass_kernel_spmd(nc, in_maps, list(range(NCORES)))
    return assemble_output(res.results)



# revision 22
# speedup vs baseline: 13.3052x; 13.3052x over previous
"""Trainium2 Bass kernel for a 5-layer bidirectional GRU (T=256, B=128, I=128, H=512, O=1).

Strategy:
  - Data-parallel over batch: 8 cores x 16 batch elements (SPMD, no collectives).
  - Everything feature-major [feature, token] so no transposes are ever needed:
      * recurrent matmul is weight-stationary: gh[3H, B] = sum_k WhhT[k].T @ h[k]
      * input-side gates gx[3H, tok] precomputed per 32-step block with big
        token-parallel matmuls (weights stationary across 512 token columns)
  - bf16 matmuls (fp32 is 4x slower on the PE), fp32 PSUM accumulate, fp32
    master hidden state, bf16 activations.
  - Both directions interleaved per step: two independent dependency chains
    keep the PE busy while the other direction's elementwise runs.
  - Activations ping-pong through internal DRAM between layers.
"""

import sys

sys.path.insert(0, "/opt/trn_rl_repo")

import numpy as np
import ml_dtypes

import concourse.bass as bass
import concourse.bacc as bacc
import concourse.mybir as mybir
import concourse.tile as tile
from concourse.vector_clock import ScopedClock, VectorClock
from concourse.bass_utils import run_bass_kernel_spmd

BF16 = mybir.dt.bfloat16
F32 = mybir.dt.float32
AF = mybir.ActivationFunctionType
OP = mybir.AluOpType

T, B, I, H, O, L = 256, 128, 128, 512, 1, 5
G3 = 3 * H  # 1536
NCORES = 8
BC = B // NCORES            # 16 batch per core
TOK = T * BC                # 4096 token columns per core
NBLK = T // 32              # 8 blocks of 32 timesteps
KH = H // 128               # 4 k-chunks of the hidden dim
M3 = G3 // 128              # 12 m-chunks of the gate dim
P = 128


class ChunkedDrainTC(tile.TileContext):
    """Work around walrus's 2-sync-wait limit on the kernel-tail drain by
    splitting the final drain into several drains with <=2 waits each."""

    def _drain_and_barrier(self, tick_clock, wait_clock):
        gc = tick_clock.global_clock
        n = len(gc)
        for i0 in range(0, n, 2):
            vec = [0] * n
            any_set = False
            for i in range(i0, min(i0 + 2, n)):
                vec[i] = gc[i]
                any_set = any_set or gc[i] > 0
            if not any_set:
                continue
            di = self.nc.sync.drain()
            wait_clock.add_sem_waits(di.ins, ScopedClock({None: VectorClock(vec)}))
        self.nc.all_engine_barrier()
        popped = self.nc._tile_sem_poison_stack.pop()
        assert popped is self._sem_poison
        self.nc.clear_and_free_semaphores(list(self.sems.allocated().values()))
        self.nc.all_engine_barrier()


def thin_pe_sems(nc):
    """Post-finalize pass: drop PE-semaphore increments whose tick value is
    never awaited, renumbering every wait on that semaphore.

    Tile's vector clock gives every PE instruction a `sem-inc`; consumers
    wait with `sem-ge-imm <abs tick>`. With in-order engines, only ticks that
    are actually awaited need an increment. Each inc costs ~26ns serialized
    on the PE, ~2.1us/step here. Renumbering `v -> rank(v among kept)` is
    exact: the kept update with that rank IS the original v-th update.
    """
    f = nc.m.functions[0]
    all_ins = [i for b in f.blocks for i in b.instructions]

    # ordered updates per sem (engines execute their stream in program order)
    upd_insts = {}   # sem -> list of (inst, update_obj) in tick order
    for i in all_ins:
        si = i.sync_info
        if si is None:
            continue
        for u in si.on_update:
            if u.update_mode == "sem-inc" and u.update_value == 1:
                upd_insts.setdefault(u.ant_name, []).append(i)

    # all awaited values per sem
    waited = {}
    for i in all_ins:
        si = i.sync_info
        if si is None:
            continue
        for w in si.on_wait:
            assert w.wait_reg is None
            if w.wait_mode == "sem-ge-imm":
                waited.setdefault(w.ant_name, set()).add(w.wait_value)

    for sem, insts in upd_insts.items():
        if not sem.startswith("PE_"):
            continue
        need = waited.get(sem, set())
        keep = [False] * (len(insts) + 1)  # 1-indexed ticks
        for v in need:
            assert 1 <= v <= len(insts), (sem, v, len(insts))
            keep[v] = True
        # rank[v] = kept count among ticks 1..v
        rank = [0] * (len(insts) + 1)
        acc = 0
        for v in range(1, len(insts) + 1):
            if keep[v]:
                acc += 1
            rank[v] = acc
        import concourse.mybir as _mybir
        stripped = 0
        for tick, inst in enumerate(insts, start=1):
            if keep[tick] or type(inst).__name__ != "InstMatmult":
                continue
            si = inst.sync_info
            new_upd = [u for u in si.on_update if u.ant_name != sem]
            inst.sync_info = _mybir.SyncInfo(
                on_wait=list(si.on_wait), on_update=new_upd
            )
            stripped += 1
        if stripped:
            # renumber all waits on this sem (stripped ticks were never
            # awaited, so every awaited v maps to its kept-rank exactly)
            for i in all_ins:
                si = i.sync_info
                if si is None or not si.on_wait:
                    continue
                changed = False
                wl = list(si.on_wait)
                for w in wl:
                    if w.wait_mode == "sem-ge-imm" and w.ant_name == sem:
                        nv = rank[w.wait_value]
                        if nv != w.wait_value:
                            w.wait_value = nv
                            changed = True
                if changed:
                    i.sync_info = _mybir.SyncInfo(
                        on_wait=wl, on_update=list(si.on_update)
                    )


def build_bass(t_steps=T, n_layers=L, repeat=1, elem_mode="full", whh_fp8=True,
               thin_sems=True, IDMM=False):
    """Build the SPMD per-core program. Returns nc.

    repeat > 1 re-runs the whole network that many times (for timing by
    differencing out the fixed dispatch overhead)."""
    nblk = t_steps // 32
    tok = t_steps * BC
    WHH_DT = mybir.dt.float8e4 if whh_fp8 else BF16

    nc = bacc.Bacc(None)

    # ---- external I/O ----
    x_in = nc.dram_tensor("x", [1, P, tok], BF16, kind="ExternalInput")
    ident_in = nc.dram_tensor("ident", [P, P], BF16, kind="ExternalInput")
    wih0_in = nc.dram_tensor("wih0", [2, I, G3], BF16, kind="ExternalInput")
    whh_in = nc.dram_tensor("whh", [n_layers, 2, H, G3], WHH_DT, kind="ExternalInput")
    gxb_in = nc.dram_tensor("gxb", [n_layers, 2, P, M3], F32, kind="ExternalInput")
    bhnb_in = nc.dram_tensor("bhnb", [n_layers, 2, 1, H], BF16, kind="ExternalInput")
    out_d = nc.dram_tensor("out", [1, tok], F32, kind="ExternalOutput")
    if n_layers > 1:
        wih_in = nc.dram_tensor(
            "wih", [n_layers - 1, 2, 2 * H, G3], BF16, kind="ExternalInput"
        )
    fcw_in = nc.dram_tensor("fcw", [2 * H, 1], BF16, kind="ExternalInput")
    fcb_in = nc.dram_tensor("fcb", [1, 1], F32, kind="ExternalInput")

    # ---- internal DRAM activation ping-pong [k-chunk, 128, tok] ----
    act_a = nc.dram_tensor("act_a", [2 * KH, P, tok], BF16)
    act_b = nc.dram_tensor("act_b", [2 * KH, P, tok], BF16)

    with ChunkedDrainTC(nc) as tc:
        with (
            tc.tile_pool(name="wpool", bufs=1) as wpool,        # weights
            tc.tile_pool(name="state", bufs=1) as state,        # per-layer state
            tc.tile_pool(name="stage", bufs=3) as stage_pool,   # act staging
            tc.tile_pool(name="tmp", bufs=4) as tmp_pool,       # elementwise temps
            tc.tile_pool(name="ghps", bufs=1, space="PSUM") as ghps_pool,
            tc.tile_pool(name="gxps", bufs=3, space="PSUM") as gxps_pool,
        ):
            ident_sb = wpool.tile([P, P], BF16, tag="ident")
            nc.sync.dma_start(ident_sb[:], ident_in[:])
            for layer in [ly for _ in range(repeat) for ly in range(n_layers)]:
                ki = 1 if layer == 0 else 2 * KH  # input k-chunks
                act_in = x_in if layer == 0 else (act_a if layer % 2 == 1 else act_b)
                act_out = act_a if layer % 2 == 0 else act_b

                # ---- load weights/biases for both dirs ----
                whh_sb, wih_sb, gxb_sb, bhn_sb = [], [], [], []
                for d in range(2):
                    w = wpool.tile([P, KH, G3], WHH_DT, tag=f"whh{d}")
                    nc.sync.dma_start(
                        w[:], whh_in[layer, d].rearrange("(ko p) m -> p ko m", p=P)
                    )
                    whh_sb.append(w)
                    wi = wpool.tile([P, ki, G3], BF16, tag=f"wih{d}")
                    src = (
                        wih0_in[d]
                        if layer == 0
                        else wih_in[layer - 1, d]
                    ).rearrange("(ko p) m -> p ko m", p=P)
                    nc.sync.dma_start(wi[:], src)
                    wih_sb.append(wi)
                    gb = wpool.tile([P, M3], F32, tag=f"gxb{d}")
                    nc.sync.dma_start(gb[:], gxb_in[layer, d])
                    gxb_sb.append(gb)
                    bh = wpool.tile([1, H], BF16, tag=f"bhnb{d}")
                    nc.sync.dma_start(bh[:], bhnb_in[layer, d])
                    bhn_sb.append(bh)
                ones_sb = wpool.tile([1, BC], BF16, tag="ones")
                nc.vector.memset(ones_sb[:], 1.0)

                # ---- per-layer state ----
                h_hist = []     # bf16 hidden history ring [128, KH, 2, 32*BC]
                gx_ring = []    # bf16 input-gate ring  [128, M3, 2, 32*BC]
                for d in range(2):
                    hh = state.tile([P, KH, 2, 32 * BC], BF16, tag=f"hh{d}")
                    nc.vector.memset(hh[:], 0.0)
                    h_hist.append(hh)
                    gxr = state.tile([P, M3, 2, 32 * BC], BF16, tag=f"gx{d}", name=f"gx{d}")
                    gx_ring.append(gxr)

                stage_sb = [None, None]

                def emit_gx_stage(d, tb):
                    """DMA the act tokens of t-block tb into SBUF staging."""
                    st = stage_pool.tile([P, ki, 32 * BC], BF16, tag="stage", name="st")
                    nc.sync.dma_start(
                        st[:],
                        act_in[0:ki, :, tb * 32 * BC : (tb + 1) * 32 * BC].rearrange(
                            "k p c -> p k c"
                        ),
                    )
                    stage_sb[d] = st

                def emit_gx_group(d, tb, m):
                    """Input-side gate matmuls for one m-chunk of t-block tb."""
                    par = tb % 2
                    st = stage_sb[d]
                    ps = gxps_pool.tile([P, 32 * BC], F32, tag="gxps", name="gxps")
                    for k in range(ki):
                        nc.tensor.matmul(
                            ps[:],
                            wih_sb[d][:, k, m * P : (m + 1) * P],
                            st[:, k, :],
                            start=(k == 0),
                            stop=(k == ki - 1),
                        )
                    # copy psum -> ring with per-feature bias, f32 -> bf16
                    nc.scalar.activation(
                        gx_ring[d][:, m, par, :],
                        ps[:],
                        AF.Identity,
                        bias=gxb_sb[d][:, m : m + 1],
                    )

                def emit_flush(d, tb):
                    """Store finished hidden states of t-block tb to DRAM act."""
                    par = tb % 2
                    nc.sync.dma_start(
                        act_out[
                            d * KH : (d + 1) * KH, :, tb * 32 * BC : (tb + 1) * 32 * BC
                        ].rearrange("k p c -> p k c"),
                        h_hist[d][:, :, par, :],
                    )

                def emit_step(d, t, ghrz, ghn):
                    """Recurrent matmuls for dir d. n-gate chunks go first
                    (own PSUM bank, ready early for the n path); rz chunks
                    next (own bank), then one identity matmul folds the
                    precomputed gx_rz into the rz bank so sigmoid can read
                    PSUM directly."""
                    td = t if d == 0 else (t_steps - 1 - t)  # token this step computes
                    prev = td - 1 if d == 0 else td + 1      # token holding h_{prev}
                    slp, pap = prev % 32, (prev // 32) % 2
                    sl, pa = td % 32, (td // 32) % 2

                    rhs = h_hist[d][:, :, pap, slp * BC : (slp + 1) * BC]
                    for m in range(8, M3):  # n chunks + b_hn bias matmul
                        sl_ = slice((m - 8) * BC, (m - 7) * BC)
                        for k in range(KH):
                            nc.tensor.matmul(
                                ghn[:, sl_],
                                whh_sb[d][:, k, m * P : (m + 1) * P],
                                rhs[:, k, :],
                                start=(k == 0),
                                stop=False,
                            )
                        nc.tensor.matmul(
                            ghn[:, sl_],
                            bhn_sb[d][0:1, (m - 8) * P : (m - 7) * P],
                            ones_sb[0:1, :],
                            start=False,
                            stop=True,
                        )
                    for m in range(8):  # r,z chunks
                        sl_ = slice(m * BC, (m + 1) * BC)
                        for k in range(KH):
                            nc.tensor.matmul(
                                ghrz[:, sl_],
                                whh_sb[d][:, k, m * P : (m + 1) * P],
                                rhs[:, k, :],
                                start=(k == 0),
                                stop=(k == KH - 1),
                            )
                    if not IDMM:
                        return
                    nc.tensor.matmul(
                        ghrz[:, 0 : 8 * BC],
                        ident_sb[:],
                        gx_ring[d][:, 0:8, pa, sl * BC : (sl + 1) * BC],
                        start=False,
                        stop=True,
                        skip_group_check=True,
                    )

                def emit_elem(d, t, ghrz, ghn):
                    """Gate math for one direction. Critical chain is
                    sigma_r -> r*hn -> +gx_n -> tanh -> *(1-z) -> +z*h_prev;
                    the z-side products are computed off-path during the tanh
                    window. 5 DVE + 4 ACT ops, bf16 temps."""
                    rzv = ghrz[:, 0 : 8 * BC].rearrange("p (m b) -> p m b", b=BC)
                    nv = ghn[:, 0 : KH * BC].rearrange("p (m b) -> p m b", b=BC)
                    td = t if d == 0 else (t_steps - 1 - t)
                    sl, pa = td % 32, (td // 32) % 2
                    prev = td - 1 if d == 0 else td + 1
                    slp, pap = prev % 32, (prev // 32) % 2
                    h_new = h_hist[d][:, :, pa, sl * BC : (sl + 1) * BC]
                    if elem_mode == "dummy":
                        nc.vector.tensor_copy(h_new, nv[:, 0:KH, :])
                        return
                    gx_n = gx_ring[d][:, 8:12, pa, sl * BC : (sl + 1) * BC]
                    h_prev = h_hist[d][:, :, pap, slp * BC : (slp + 1) * BC]

                    if not IDMM:
                        gx_rz = gx_ring[d][:, 0:8, pa, sl * BC : (sl + 1) * BC]
                        trz = tmp_pool.tile([P, 8, BC], BF16, tag=f"trz{d}", name="trz")
                        nc.vector.tensor_add(trz[:], rzv[:], gx_rz)
                        rzin = trz[:]
                    else:
                        rzin = rzv
                    r_t = tmp_pool.tile([P, KH, BC], BF16, tag=f"r{d}", name="r")
                    nc.scalar.activation(r_t[:], rzin[:, 0:KH, :], AF.Sigmoid)
                    z_t = tmp_pool.tile([P, KH, BC], BF16, tag=f"z{d}", name="z")
                    nc.scalar.activation(z_t[:], rzin[:, KH : 2 * KH, :], AF.Sigmoid)
                    tn = tmp_pool.tile([P, KH, BC], BF16, tag=f"tn{d}", name="tn")
                    nc.vector.tensor_mul(tn[:], nv[:], r_t[:])
                    tn2 = tmp_pool.tile([P, KH, BC], BF16, tag=f"tn2{d}", name="tn2")
                    nc.vector.tensor_add(tn2[:], tn[:], gx_n)
                    nt = tmp_pool.tile([P, KH, BC], BF16, tag=f"nt{d}", name="nt")
                    nc.scalar.activation(nt[:], tn2[:], AF.Tanh)
                    # h = n + z*(h_prev - n): rounding error scales with
                    # |h_prev - n|, not |h| -- the carried state does not
                    # accumulate multiplicative rounding drift
                    dt_ = tmp_pool.tile([P, KH, BC], BF16, tag=f"dt{d}", name="dt")
                    nc.vector.tensor_sub(dt_[:], h_prev, nt[:])
                    dt2 = tmp_pool.tile([P, KH, BC], BF16, tag=f"dt2{d}", name="dt2")
                    nc.vector.tensor_mul(dt2[:], dt_[:], z_t[:])
                    nc.vector.tensor_add(h_new, nt[:], dt2[:])

                # gx schedule: 24 m-groups (both dirs) spread over a block's 32
                # steps; group g of the NEXT block is emitted during step
                # ts where cumulative quota passes g.
                def gx_groups_due(ts):
                    lo = (ts * 24) // 32
                    hi = ((ts + 1) * 24) // 32
                    return range(lo, hi)

                # ---- prologue: stage + gx for the first consumed blocks ----
                emit_gx_stage(0, 0)
                emit_gx_stage(1, nblk - 1)
                for m in range(M3):
                    emit_gx_group(0, 0, m)
                    emit_gx_group(1, nblk - 1, m)
                for tb in range(nblk):
                    if tb < nblk - 1:
                        emit_gx_stage(0, tb + 1)
                        emit_gx_stage(1, nblk - 2 - tb)
                    for ts in range(32):
                        t = tb * 32 + ts
                        # separate PSUM banks per (dir, gate-group) so each
                        # consumer only waits for its own producer matmuls
                        grz0 = ghps_pool.tile([P, 512], F32, tag="grz0", name="grz0")
                        gn0 = ghps_pool.tile([P, 512], F32, tag="gn0", name="gn0")
                        grz1 = ghps_pool.tile([P, 512], F32, tag="grz1", name="grz1")
                        gn1 = ghps_pool.tile([P, 512], F32, tag="gn1", name="gn1")
                        emit_step(0, t, grz0, gn0)
                        emit_step(1, t, grz1, gn1)
                        if tb < nblk - 1:
                            for g in gx_groups_due(ts):
                                d, m = g % 2, g // 2
                                emit_gx_group(d, tb + 1 if d == 0 else nblk - 2 - tb, m)
                        emit_elem(0, t, grz0, gn0)
                        emit_elem(1, t, grz1, gn1)
                    emit_flush(0, tb)
                    emit_flush(1, nblk - 1 - tb)

            # ---- final FC + sigmoid over the last layer's output ----
            act_fin = act_a if (n_layers - 1) % 2 == 0 else act_b
            fcw_sb = wpool.tile([P, 2 * KH, 1], BF16, tag="fcw")
            nc.sync.dma_start(fcw_sb[:], fcw_in.rearrange("(ko p) n -> p ko n", p=P))
            fcb_sb = wpool.tile([1, 1], F32, tag="fcb")
            nc.sync.dma_start(fcb_sb[:], fcb_in[:])
            out_sb = state.tile([1, tok], F32, tag="osb")
            for blk in range(nblk):
                st = stage_pool.tile([P, 2 * KH, 32 * BC], BF16, tag="stage")
                nc.sync.dma_start(
                    st[:],
                    act_fin[:, :, blk * 32 * BC : (blk + 1) * 32 * BC].rearrange(
                        "k p c -> p k c"
                    ),
                )
                ps = gxps_pool.tile([1, 32 * BC], F32, tag="gxps")
                for k in range(2 * KH):
                    nc.tensor.matmul(
                        ps[:],
                        fcw_sb[:, k, :],
                        st[:, k, :],
                        start=(k == 0),
                        stop=(k == 2 * KH - 1),
                    )
                nc.scalar.activation(
                    out_sb[:, blk * 32 * BC : (blk + 1) * 32 * BC],
                    ps[:],
                    AF.Sigmoid,
                    bias=fcb_sb[:, 0:1],
                )
            nc.sync.dma_start(out_d[:], out_sb[:])

    nc.finalize()
    if thin_sems:
        thin_pe_sems(nc)
    return nc


def prep_inputs(input_seq, W_ih0, W_hh0, b_ih0, b_hh0, W_ih, W_hh, b_ih, b_hh,
                fc_w, fc_b, t_steps=T, n_layers=L, whh_fp8=True):
    """Host-side prep: transposes, bias folding, bf16 casts. Returns in_maps."""
    bf = ml_dtypes.bfloat16
    whh_dt = ml_dtypes.float8_e4m3fn if whh_fp8 else bf
    tok = t_steps * BC

    wih0 = np.ascontiguousarray(np.transpose(np.asarray(W_ih0), (0, 2, 1))).astype(bf)
    whh_all = np.concatenate(
        [np.asarray(W_hh0)[None], np.asarray(W_hh)], axis=0
    )[:n_layers]
    whh = np.ascontiguousarray(np.transpose(whh_all, (0, 1, 3, 2))).astype(whh_dt)
    bih_all = np.concatenate([np.asarray(b_ih0)[None], np.asarray(b_ih)], axis=0)[:n_layers]
    bhh_all = np.concatenate([np.asarray(b_hh0)[None], np.asarray(b_hh)], axis=0)[:n_layers]

    # gx bias: b_ih everywhere + b_hh on the r,z gates only (b_hn rides separately)
    gxb = bih_all.copy()
    gxb[:, :, : 2 * H] += bhh_all[:, :, : 2 * H]
    gxb = np.ascontiguousarray(
        np.transpose(gxb.reshape(n_layers, 2, M3, P), (0, 1, 3, 2))
    ).astype(np.float32)
    bhnb = np.ascontiguousarray(bhh_all[:, :, None, 2 * H :]).astype(bf)

    base = {
        "wih0": wih0,
        "whh": whh,
        "gxb": gxb,
        "bhnb": bhnb,
        "ident": np.eye(P, dtype=bf),
        "fcw": np.ascontiguousarray(np.asarray(fc_w).T).astype(bf),
        "fcb": np.asarray(fc_b, dtype=np.float32).reshape(1, 1),
    }
    if n_layers > 1:
        base["wih"] = np.ascontiguousarray(
            np.transpose(np.asarray(W_ih), (0, 1, 3, 2))
        )[: n_layers - 1].astype(bf)

    x = np.asarray(input_seq)[:t_steps]
    in_maps = []
    for c in range(NCORES):
        xc = x[:, c * BC : (c + 1) * BC, :].reshape(tok, I).T  # [128, tok]
        m = dict(base)
        m["x"] = np.ascontiguousarray(xc)[None].astype(bf)
        in_maps.append(m)
    return in_maps


def assemble_output(results, t_steps=T):
    """results: list of per-core dicts with 'out' [1, tok] -> [T, B, 1] f32."""
    outs = []
    for c in range(NCORES):
        o = np.asarray(results[c]["out"]).reshape(t_steps, BC)
        outs.append(o)
    return np.stack(outs, axis=1).reshape(t_steps, B)[:, :, None].astype(np.float32)


def kernel(**inputs):
    nc = build_bass()
    in_maps = prep_inputs(**inputs)
    res = run_bass_kernel_spmd(nc, in_maps, list(range(NCORES)))
    return assemble_output(res.results)



# revision 27
# speedup vs baseline: 13.3789x; 1.0055x over previous
"""Trainium2 Bass kernel for a 5-layer bidirectional GRU (T=256, B=128, I=128, H=512, O=1).

Strategy:
  - Data-parallel over batch: 8 cores x 16 batch elements (SPMD, no collectives).
  - Everything feature-major [feature, token] so no transposes are ever needed:
      * recurrent matmul is weight-stationary: gh[3H, B] = sum_k WhhT[k].T @ h[k]
      * input-side gates gx[3H, tok] precomputed per 32-step block with big
        token-parallel matmuls (weights stationary across 512 token columns)
  - Whh in fp8(e4m3): the recurrent step is LDWEIGHTS-streaming-bound
    (48 [128x128] weight tiles per dir per step, free dim only 16), and FWL
    loads fp8 at 4 cols/cycle vs bf16's 2. Everything else stays bf16;
    PSUM accumulates fp32. Measured full-net rel err ~1.5e-3.
  - Per-step elementwise in bf16, per-direction chains (6 DVE + 3 ACT ops),
    with each (dir, gate-group) in its own PSUM bank so a consumer only
    waits on its own producer matmuls. h update uses n + z*(h_prev - n)
    (rounding scales with |h_prev - n|, not |h| -- no carry drift).
  - Both directions interleaved per step: while one direction's gate chain
    runs on DVE/ACT, the other direction's matmuls run on the PE.
  - Activations ping-pong through internal DRAM between layers.
  - thin_pe_sems: post-finalize pass dropping un-awaited per-matmul
    semaphore increments (tile emits one per instruction).

True device time ~9.2ms (vs 17.7ms for the f32-elementwise bf16-Whh
baseline), measured by chained-dispatch slope (see test.py; single-call
wall time is dominated by 40-90ms of axon tunnel latency).
"""

import sys

sys.path.insert(0, "/opt/trn_rl_repo")

import numpy as np
import ml_dtypes

import concourse.bass as bass
import concourse.bacc as bacc
import concourse.mybir as mybir
import concourse.tile as tile
from concourse.vector_clock import ScopedClock, VectorClock
from concourse.bass_utils import run_bass_kernel_spmd

BF16 = mybir.dt.bfloat16
F32 = mybir.dt.float32
AF = mybir.ActivationFunctionType
OP = mybir.AluOpType

T, B, I, H, O, L = 256, 128, 128, 512, 1, 5
G3 = 3 * H  # 1536
NCORES = 8
BC = B // NCORES            # 16 batch per core
TOK = T * BC                # 4096 token columns per core
NBLK = T // 32              # 8 blocks of 32 timesteps
KH = H // 128               # 4 k-chunks of the hidden dim
M3 = G3 // 128              # 12 m-chunks of the gate dim
P = 128


class ChunkedDrainTC(tile.TileContext):
    """Work around walrus's 2-sync-wait limit on the kernel-tail drain by
    splitting the final drain into several drains with <=2 waits each."""

    def _drain_and_barrier(self, tick_clock, wait_clock):
        gc = tick_clock.global_clock
        n = len(gc)
        for i0 in range(0, n, 2):
            vec = [0] * n
            any_set = False
            for i in range(i0, min(i0 + 2, n)):
                vec[i] = gc[i]
                any_set = any_set or gc[i] > 0
            if not any_set:
                continue
            di = self.nc.sync.drain()
            wait_clock.add_sem_waits(di.ins, ScopedClock({None: VectorClock(vec)}))
        self.nc.all_engine_barrier()
        popped = self.nc._tile_sem_poison_stack.pop()
        assert popped is self._sem_poison
        self.nc.clear_and_free_semaphores(list(self.sems.allocated().values()))
        self.nc.all_engine_barrier()


def thin_pe_sems(nc):
    """Post-finalize pass: drop PE-semaphore increments whose tick value is
    never awaited, renumbering every wait on that semaphore.

    Tile's vector clock gives every PE instruction a `sem-inc`; consumers
    wait with `sem-ge-imm <abs tick>`. With in-order engines, only ticks that
    are actually awaited need an increment. Each inc costs ~26ns serialized
    on the PE, ~2.1us/step here. Renumbering `v -> rank(v among kept)` is
    exact: the kept update with that rank IS the original v-th update.
    """
    f = nc.m.functions[0]
    all_ins = [i for b in f.blocks for i in b.instructions]

    # ordered updates per sem (engines execute their stream in program order)
    upd_insts = {}   # sem -> list of (inst, update_obj) in tick order
    for i in all_ins:
        si = i.sync_info
        if si is None:
            continue
        for u in si.on_update:
            if u.update_mode == "sem-inc" and u.update_value == 1:
                upd_insts.setdefault(u.ant_name, []).append(i)

    # all awaited values per sem
    waited = {}
    for i in all_ins:
        si = i.sync_info
        if si is None:
            continue
        for w in si.on_wait:
            assert w.wait_reg is None
            if w.wait_mode == "sem-ge-imm":
                waited.setdefault(w.ant_name, set()).add(w.wait_value)

    for sem, insts in upd_insts.items():
        if not sem.startswith("PE_"):
            continue
        need = waited.get(sem, set())
        keep = [False] * (len(insts) + 1)  # 1-indexed ticks
        for v in need:
            assert 1 <= v <= len(insts), (sem, v, len(insts))
            keep[v] = True
        # rank[v] = kept count among ticks 1..v
        rank = [0] * (len(insts) + 1)
        acc = 0
        for v in range(1, len(insts) + 1):
            if keep[v]:
                acc += 1
            rank[v] = acc
        import concourse.mybir as _mybir
        stripped = 0
        for tick, inst in enumerate(insts, start=1):
            if keep[tick] or type(inst).__name__ != "InstMatmult":
                continue
            si = inst.sync_info
            new_upd = [u for u in si.on_update if u.ant_name != sem]
            inst.sync_info = _mybir.SyncInfo(
                on_wait=list(si.on_wait), on_update=new_upd
            )
            stripped += 1
        if stripped:
            # renumber all waits on this sem (stripped ticks were never
            # awaited, so every awaited v maps to its kept-rank exactly)
            for i in all_ins:
                si = i.sync_info
                if si is None or not si.on_wait:
                    continue
                changed = False
                wl = list(si.on_wait)
                for w in wl:
                    if w.wait_mode == "sem-ge-imm" and w.ant_name == sem:
                        nv = rank[w.wait_value]
                        if nv != w.wait_value:
                            w.wait_value = nv
                            changed = True
                if changed:
                    i.sync_info = _mybir.SyncInfo(
                        on_wait=wl, on_update=list(si.on_update)
                    )


def build_bass(t_steps=T, n_layers=L, repeat=1, elem_mode="full", whh_fp8=True,
               thin_sems=True, IDMM=False):
    """Build the SPMD per-core program. Returns nc.

    repeat > 1 re-runs the whole network that many times (for timing by
    differencing out the fixed dispatch overhead)."""
    nblk = t_steps // 32
    tok = t_steps * BC
    WHH_DT = mybir.dt.float8e4 if whh_fp8 else BF16

    nc = bacc.Bacc(None)

    # ---- external I/O ----
    x_in = nc.dram_tensor("x", [1, P, tok], BF16, kind="ExternalInput")
    ident_in = nc.dram_tensor("ident", [P, P], BF16, kind="ExternalInput")
    wih0_in = nc.dram_tensor("wih0", [2, I, G3], BF16, kind="ExternalInput")
    whh_in = nc.dram_tensor("whh", [n_layers, 2, H, G3], WHH_DT, kind="ExternalInput")
    gxb_in = nc.dram_tensor("gxb", [n_layers, 2, P, M3], F32, kind="ExternalInput")
    bhnb_in = nc.dram_tensor("bhnb", [n_layers, 2, 1, H], BF16, kind="ExternalInput")
    out_d = nc.dram_tensor("out", [1, tok], F32, kind="ExternalOutput")
    if n_layers > 1:
        wih_in = nc.dram_tensor(
            "wih", [n_layers - 1, 2, 2 * H, G3], BF16, kind="ExternalInput"
        )
    fcw_in = nc.dram_tensor("fcw", [2 * H, 1], BF16, kind="ExternalInput")
    fcb_in = nc.dram_tensor("fcb", [1, 1], F32, kind="ExternalInput")

    # ---- internal DRAM activation ping-pong [k-chunk, 128, tok] ----
    act_a = nc.dram_tensor("act_a", [2 * KH, P, tok], BF16)
    act_b = nc.dram_tensor("act_b", [2 * KH, P, tok], BF16)

    with ChunkedDrainTC(nc) as tc:
        with (
            tc.tile_pool(name="wpool", bufs=1) as wpool,        # weights
            tc.tile_pool(name="state", bufs=1) as state,        # per-layer state
            tc.tile_pool(name="stage", bufs=3) as stage_pool,   # act staging
            tc.tile_pool(name="tmp", bufs=4) as tmp_pool,       # elementwise temps
            tc.tile_pool(name="ghps", bufs=1, space="PSUM") as ghps_pool,
            tc.tile_pool(name="gxps", bufs=3, space="PSUM") as gxps_pool,
        ):
            ident_sb = wpool.tile([P, P], BF16, tag="ident")
            nc.sync.dma_start(ident_sb[:], ident_in[:])
            for layer in [ly for _ in range(repeat) for ly in range(n_layers)]:
                ki = 1 if layer == 0 else 2 * KH  # input k-chunks
                act_in = x_in if layer == 0 else (act_a if layer % 2 == 1 else act_b)
                act_out = act_a if layer % 2 == 0 else act_b

                # ---- load weights/biases for both dirs ----
                whh_sb, wih_sb, gxb_sb, bhn_sb = [], [], [], []
                for d in range(2):
                    w = wpool.tile([P, KH, G3], WHH_DT, tag=f"whh{d}")
                    nc.sync.dma_start(
                        w[:], whh_in[layer, d].rearrange("(ko p) m -> p ko m", p=P)
                    )
                    whh_sb.append(w)
                    wi = wpool.tile([P, ki, G3], BF16, tag=f"wih{d}")
                    src = (
                        wih0_in[d]
                        if layer == 0
                        else wih_in[layer - 1, d]
                    ).rearrange("(ko p) m -> p ko m", p=P)
                    nc.sync.dma_start(wi[:], src)
                    wih_sb.append(wi)
                    gb = wpool.tile([P, M3], F32, tag=f"gxb{d}")
                    nc.sync.dma_start(gb[:], gxb_in[layer, d])
                    gxb_sb.append(gb)
                    bh = wpool.tile([1, H], BF16, tag=f"bhnb{d}")
                    nc.sync.dma_start(bh[:], bhnb_in[layer, d])
                    bhn_sb.append(bh)
                ones_sb = wpool.tile([1, BC], BF16, tag="ones")
                nc.vector.memset(ones_sb[:], 1.0)

                # ---- per-layer state ----
                h_hist = []     # bf16 hidden history ring [128, KH, 2, 32*BC]
                gx_ring = []    # bf16 input-gate ring  [128, M3, 2, 32*BC]
                for d in range(2):
                    hh = state.tile([P, KH, 2, 32 * BC], BF16, tag=f"hh{d}")
                    nc.vector.memset(hh[:], 0.0)
                    h_hist.append(hh)
                    gxr = state.tile([P, M3, 2, 32 * BC], BF16, tag=f"gx{d}", name=f"gx{d}")
                    gx_ring.append(gxr)

                stage_sb = [None, None]

                def emit_gx_stage(d, tb):
                    """DMA the act tokens of t-block tb into SBUF staging."""
                    st = stage_pool.tile([P, ki, 32 * BC], BF16, tag="stage", name="st")
                    nc.sync.dma_start(
                        st[:],
                        act_in[0:ki, :, tb * 32 * BC : (tb + 1) * 32 * BC].rearrange(
                            "k p c -> p k c"
                        ),
                    )
                    stage_sb[d] = st

                def emit_gx_group(d, tb, m):
                    """Input-side gate matmuls for one m-chunk of t-block tb."""
                    par = tb % 2
                    st = stage_sb[d]
                    ps = gxps_pool.tile([P, 32 * BC], F32, tag="gxps", name="gxps")
                    for k in range(ki):
                        nc.tensor.matmul(
                            ps[:],
                            wih_sb[d][:, k, m * P : (m + 1) * P],
                            st[:, k, :],
                            start=(k == 0),
                            stop=(k == ki - 1),
                        )
                    # copy psum -> ring with per-feature bias, f32 -> bf16
                    nc.scalar.activation(
                        gx_ring[d][:, m, par, :],
                        ps[:],
                        AF.Identity,
                        bias=gxb_sb[d][:, m : m + 1],
                    )

                def emit_flush(d, tb):
                    """Store finished hidden states of t-block tb to DRAM act."""
                    par = tb % 2
                    nc.sync.dma_start(
                        act_out[
                            d * KH : (d + 1) * KH, :, tb * 32 * BC : (tb + 1) * 32 * BC
                        ].rearrange("k p c -> p k c"),
                        h_hist[d][:, :, par, :],
                    )

                def emit_step(d, t, ghrz, ghn):
                    """Recurrent matmuls for dir d. n-gate chunks go first
                    (own PSUM bank, ready early for the n path); rz chunks
                    next (own bank), then one identity matmul folds the
                    precomputed gx_rz into the rz bank so sigmoid can read
                    PSUM directly."""
                    td = t if d == 0 else (t_steps - 1 - t)  # token this step computes
                    prev = td - 1 if d == 0 else td + 1      # token holding h_{prev}
                    slp, pap = prev % 32, (prev // 32) % 2
                    sl, pa = td % 32, (td // 32) % 2

                    rhs = h_hist[d][:, :, pap, slp * BC : (slp + 1) * BC]
                    for m in range(8, M3):  # n chunks + b_hn bias matmul
                        sl_ = slice((m - 8) * BC, (m - 7) * BC)
                        for k in range(KH):
                            nc.tensor.matmul(
                                ghn[:, sl_],
                                whh_sb[d][:, k, m * P : (m + 1) * P],
                                rhs[:, k, :],
                                start=(k == 0),
                                stop=False,
                            )
                        nc.tensor.matmul(
                            ghn[:, sl_],
                            bhn_sb[d][0:1, (m - 8) * P : (m - 7) * P],
                            ones_sb[0:1, :],
                            start=False,
                            stop=True,
                        )
                    for m in range(8):  # r,z chunks
                        sl_ = slice(m * BC, (m + 1) * BC)
                        for k in range(KH):
                            nc.tensor.matmul(
                                ghrz[:, sl_],
                                whh_sb[d][:, k, m * P : (m + 1) * P],
                                rhs[:, k, :],
                                start=(k == 0),
                                stop=(k == KH - 1),
                            )
                    if not IDMM:
                        return
                    nc.tensor.matmul(
                        ghrz[:, 0 : 8 * BC],
                        ident_sb[:],
                        gx_ring[d][:, 0:8, pa, sl * BC : (sl + 1) * BC],
                        start=False,
                        stop=True,
                        skip_group_check=True,
                    )

                def emit_elem_pair(t, grz, gn):
                    """Gate math for BOTH directions, phase-interleaved.

                    DVE and ACT are strict-FIFO: emitting dir f's whole chain
                    before dir b's parks b's ready ops behind f's unready
                    ones, serializing the two chains (measured C ~ P + 2E).
                    Interleaving by phase lets both chains run in parallel:
                      DVE: trz_f trz_b | tn_f tn_b tn2_f tn2_b | tail_f tail_b
                      ACT: sr_f sr_b sz_f sz_b | tanh_f tanh_b
                    h = n + z*(h_prev - n): rounding scales with |h_prev - n|,
                    not |h| -- no multiplicative drift on the carried state."""
                    rzv, nv, h_new, gx_n, gx_rz, h_prev = [], [], [], [], [], []
                    for d in range(2):
                        rzv.append(
                            grz[d][:, 0 : 8 * BC].rearrange("p (m b) -> p m b", b=BC)
                        )
                        nv.append(
                            gn[d][:, 0 : KH * BC].rearrange("p (m b) -> p m b", b=BC)
                        )
                        td = t if d == 0 else (t_steps - 1 - t)
                        sl, pa = td % 32, (td // 32) % 2
                        prev = td - 1 if d == 0 else td + 1
                        slp, pap = prev % 32, (prev // 32) % 2
                        h_new.append(h_hist[d][:, :, pa, sl * BC : (sl + 1) * BC])
                        gx_n.append(gx_ring[d][:, 8:12, pa, sl * BC : (sl + 1) * BC])
                        gx_rz.append(gx_ring[d][:, 0:8, pa, sl * BC : (sl + 1) * BC])
                        h_prev.append(
                            h_hist[d][:, :, pap, slp * BC : (slp + 1) * BC]
                        )
                    if elem_mode == "dummy":
                        for d in range(2):
                            nc.vector.tensor_copy(h_new[d], nv[d][:, 0:KH, :])
                        return

                    def tiles(tag):
                        return [
                            tmp_pool.tile([P, KH, BC], BF16, tag=f"{tag}{d}", name=tag)
                            for d in range(2)
                        ]

                    trz = [
                        tmp_pool.tile([P, 8, BC], BF16, tag=f"trz{d}", name="trz")
                        for d in range(2)
                    ]
                    rzin = []
                    for d in range(2):
                        if IDMM:
                            rzin.append(rzv[d])
                        else:
                            nc.vector.tensor_add(trz[d][:], rzv[d][:], gx_rz[d])
                            rzin.append(trz[d][:])
                    r_t, z_t = tiles("r"), tiles("z")
                    for d in range(2):
                        nc.scalar.activation(r_t[d][:], rzin[d][:, 0:KH, :], AF.Sigmoid)
                    for d in range(2):
                        nc.scalar.activation(
                            z_t[d][:], rzin[d][:, KH : 2 * KH, :], AF.Sigmoid
                        )
                    tn, tn2, nt = tiles("tn"), tiles("tn2"), tiles("nt")
                    for d in range(2):
                        nc.vector.tensor_mul(tn[d][:], nv[d][:], r_t[d][:])
                    for d in range(2):
                        nc.vector.tensor_add(tn2[d][:], tn[d][:], gx_n[d])
                    for d in range(2):
                        nc.scalar.activation(nt[d][:], tn2[d][:], AF.Tanh)
                    dt_, dt2 = tiles("dt"), tiles("dt2")
                    for d in range(2):
                        nc.vector.tensor_sub(dt_[d][:], h_prev[d], nt[d][:])
                        nc.vector.tensor_mul(dt2[d][:], dt_[d][:], z_t[d][:])
                        nc.vector.tensor_add(h_new[d], nt[d][:], dt2[d][:])

                # gx schedule: 24 m-groups (both dirs) spread over a block's 32
                # steps; group g of the NEXT block is emitted during step
                # ts where cumulative quota passes g.
                def gx_groups_due(ts):
                    lo = (ts * 24) // 32
                    hi = ((ts + 1) * 24) // 32
                    return range(lo, hi)

                # ---- prologue: stage + gx for the first consumed blocks ----
                emit_gx_stage(0, 0)
                emit_gx_stage(1, nblk - 1)
                for m in range(M3):
                    emit_gx_group(0, 0, m)
                    emit_gx_group(1, nblk - 1, m)
                for tb in range(nblk):
                    if tb < nblk - 1:
                        emit_gx_stage(0, tb + 1)
                        emit_gx_stage(1, nblk - 2 - tb)
                    for ts in range(32):
                        t = tb * 32 + ts
                        # separate PSUM banks per (dir, gate-group) so each
                        # consumer only waits for its own producer matmuls
                        grz0 = ghps_pool.tile([P, 512], F32, tag="grz0", name="grz0")
                        gn0 = ghps_pool.tile([P, 512], F32, tag="gn0", name="gn0")
                        grz1 = ghps_pool.tile([P, 512], F32, tag="grz1", name="grz1")
                        gn1 = ghps_pool.tile([P, 512], F32, tag="gn1", name="gn1")
                        emit_step(0, t, grz0, gn0)
                        emit_step(1, t, grz1, gn1)
                        if tb < nblk - 1:
                            for g in gx_groups_due(ts):
                                d, m = g % 2, g // 2
                                emit_gx_group(d, tb + 1 if d == 0 else nblk - 2 - tb, m)
                        emit_elem_pair(t, [grz0, grz1], [gn0, gn1])
                    emit_flush(0, tb)
                    emit_flush(1, nblk - 1 - tb)

            # ---- final FC + sigmoid over the last layer's output ----
            act_fin = act_a if (n_layers - 1) % 2 == 0 else act_b
            fcw_sb = wpool.tile([P, 2 * KH, 1], BF16, tag="fcw")
            nc.sync.dma_start(fcw_sb[:], fcw_in.rearrange("(ko p) n -> p ko n", p=P))
            fcb_sb = wpool.tile([1, 1], F32, tag="fcb")
            nc.sync.dma_start(fcb_sb[:], fcb_in[:])
            out_sb = state.tile([1, tok], F32, tag="osb")
            for blk in range(nblk):
                st = stage_pool.tile([P, 2 * KH, 32 * BC], BF16, tag="stage")
                nc.sync.dma_start(
                    st[:],
                    act_fin[:, :, blk * 32 * BC : (blk + 1) * 32 * BC].rearrange(
                        "k p c -> p k c"
                    ),
                )
                ps = gxps_pool.tile([1, 32 * BC], F32, tag="gxps")
                for k in range(2 * KH):
                    nc.tensor.matmul(
                        ps[:],
                        fcw_sb[:, k, :],
                        st[:, k, :],
                        start=(k == 0),
                        stop=(k == 2 * KH - 1),
                    )
                nc.scalar.activation(
                    out_sb[:, blk * 32 * BC : (blk + 1) * 32 * BC],
                    ps[:],
                    AF.Sigmoid,
                    bias=fcb_sb[:, 0:1],
                )
            nc.sync.dma_start(out_d[:], out_sb[:])

    nc.finalize()
    if thin_sems:
        thin_pe_sems(nc)
    return nc


def prep_inputs(input_seq, W_ih0, W_hh0, b_ih0, b_hh0, W_ih, W_hh, b_ih, b_hh,
                fc_w, fc_b, t_steps=T, n_layers=L, whh_fp8=True):
    """Host-side prep: transposes, bias folding, bf16 casts. Returns in_maps."""
    bf = ml_dtypes.bfloat16
    whh_dt = ml_dtypes.float8_e4m3fn if whh_fp8 else bf
    tok = t_steps * BC

    wih0 = np.ascontiguousarray(np.transpose(np.asarray(W_ih0), (0, 2, 1))).astype(bf)
    whh_all = np.concatenate(
        [np.asarray(W_hh0)[None], np.asarray(W_hh)], axis=0
    )[:n_layers]
    whh = np.ascontiguousarray(np.transpose(whh_all, (0, 1, 3, 2))).astype(whh_dt)
    bih_all = np.concatenate([np.asarray(b_ih0)[None], np.asarray(b_ih)], axis=0)[:n_layers]
    bhh_all = np.concatenate([np.asarray(b_hh0)[None], np.asarray(b_hh)], axis=0)[:n_layers]

    # gx bias: b_ih everywhere + b_hh on the r,z gates only (b_hn rides separately)
    gxb = bih_all.copy()
    gxb[:, :, : 2 * H] += bhh_all[:, :, : 2 * H]
    gxb = np.ascontiguousarray(
        np.transpose(gxb.reshape(n_layers, 2, M3, P), (0, 1, 3, 2))
    ).astype(np.float32)
    bhnb = np.ascontiguousarray(bhh_all[:, :, None, 2 * H :]).astype(bf)

    base = {
        "wih0": wih0,
        "whh": whh,
        "gxb": gxb,
        "bhnb": bhnb,
        "ident": np.eye(P, dtype=bf),
        "fcw": np.ascontiguousarray(np.asarray(fc_w).T).astype(bf),
        "fcb": np.asarray(fc_b, dtype=np.float32).reshape(1, 1),
    }
    if n_layers > 1:
        base["wih"] = np.ascontiguousarray(
            np.transpose(np.asarray(W_ih), (0, 1, 3, 2))
        )[: n_layers - 1].astype(bf)

    x = np.asarray(input_seq)[:t_steps]
    in_maps = []
    for c in range(NCORES):
        xc = x[:, c * BC : (c + 1) * BC, :].reshape(tok, I).T  # [128, tok]
        m = dict(base)
        m["x"] = np.ascontiguousarray(xc)[None].astype(bf)
        in_maps.append(m)
    return in_maps


def assemble_output(results, t_steps=T):
    """results: list of per-core dicts with 'out' [1, tok] -> [T, B, 1] f32."""
    outs = []
    for c in range(NCORES):
        o = np.asarray(results[c]["out"]).reshape(t_steps, BC)
        outs.append(o)
    return np.stack(outs, axis=1).reshape(t_steps, B)[:, :, None].astype(np.float32)


def kernel(**inputs):
    nc = build_bass()
    in_maps = prep_inputs(**inputs)
    res = run_bass_kernel_spmd(nc, in_maps, list(range(NCORES)))
    return assemble_output(res.results)



# revision 31
# speedup vs baseline: 15.8011x; 1.1810x over previous
"""Trainium2 Bass kernel for a 5-layer bidirectional GRU (T=256, B=128, I=128, H=512, O=1).

Strategy:
  - Data-parallel over batch: 8 cores x 16 batch elements (SPMD, no collectives).
  - Everything feature-major [feature, token] so no transposes are ever needed:
      * recurrent matmul is weight-stationary: gh[3H, B] = sum_k WhhT[k].T @ h[k]
      * input-side gates gx[3H, tok] precomputed per 32-step block with big
        token-parallel matmuls (weights stationary across 512 token columns)
  - Whh in fp8(e4m3): the recurrent step is LDWEIGHTS-streaming-bound
    (48 [128x128] weight tiles per dir per step, free dim only 16), and FWL
    loads fp8 at 4 cols/cycle vs bf16's 2. Everything else stays bf16;
    PSUM accumulates fp32. Measured full-net rel err ~1.5e-3.
  - Per-step elementwise in bf16, per-direction chains (6 DVE + 3 ACT ops),
    with each (dir, gate-group) in its own PSUM bank so a consumer only
    waits on its own producer matmuls. h update uses n + z*(h_prev - n)
    (rounding scales with |h_prev - n|, not |h| -- no carry drift).
  - Both directions interleaved per step: while one direction's gate chain
    runs on DVE/ACT, the other direction's matmuls run on the PE.
  - Activations ping-pong through internal DRAM between layers.
  - thin_pe_sems: post-finalize pass dropping un-awaited per-matmul
    semaphore increments (tile emits one per instruction).

True device time ~9.2ms (vs 17.7ms for the f32-elementwise bf16-Whh
baseline), measured by chained-dispatch slope (see test.py; single-call
wall time is dominated by 40-90ms of axon tunnel latency).
"""

import sys

sys.path.insert(0, "/opt/trn_rl_repo")

import numpy as np
import ml_dtypes

import concourse.bass as bass
import concourse.bacc as bacc
import concourse.mybir as mybir
import concourse.tile as tile
from concourse.vector_clock import ScopedClock, VectorClock
from concourse.bass_utils import run_bass_kernel_spmd

BF16 = mybir.dt.bfloat16
F32 = mybir.dt.float32
AF = mybir.ActivationFunctionType
OP = mybir.AluOpType

T, B, I, H, O, L = 256, 128, 128, 512, 1, 5
G3 = 3 * H  # 1536
NCORES = 8
BC = B // NCORES            # 16 batch per core
TOK = T * BC                # 4096 token columns per core
NBLK = T // 32              # 8 blocks of 32 timesteps
KH = H // 128               # 4 k-chunks of the hidden dim
M3 = G3 // 128              # 12 m-chunks of the gate dim
P = 128


class ChunkedDrainTC(tile.TileContext):
    """Work around walrus's 2-sync-wait limit on the kernel-tail drain by
    splitting the final drain into several drains with <=2 waits each."""

    def _drain_and_barrier(self, tick_clock, wait_clock):
        gc = tick_clock.global_clock
        n = len(gc)
        for i0 in range(0, n, 2):
            vec = [0] * n
            any_set = False
            for i in range(i0, min(i0 + 2, n)):
                vec[i] = gc[i]
                any_set = any_set or gc[i] > 0
            if not any_set:
                continue
            di = self.nc.sync.drain()
            wait_clock.add_sem_waits(di.ins, ScopedClock({None: VectorClock(vec)}))
        self.nc.all_engine_barrier()
        popped = self.nc._tile_sem_poison_stack.pop()
        assert popped is self._sem_poison
        self.nc.clear_and_free_semaphores(list(self.sems.allocated().values()))
        self.nc.all_engine_barrier()


def thin_pe_sems(nc):
    """Post-finalize pass: drop PE-semaphore increments whose tick value is
    never awaited, renumbering every wait on that semaphore.

    Tile's vector clock gives every PE instruction a `sem-inc`; consumers
    wait with `sem-ge-imm <abs tick>`. With in-order engines, only ticks that
    are actually awaited need an increment. Each inc costs ~26ns serialized
    on the PE, ~2.1us/step here. Renumbering `v -> rank(v among kept)` is
    exact: the kept update with that rank IS the original v-th update.
    """
    f = nc.m.functions[0]
    all_ins = [i for b in f.blocks for i in b.instructions]

    # ordered updates per sem (engines execute their stream in program order)
    upd_insts = {}   # sem -> list of (inst, update_obj) in tick order
    for i in all_ins:
        si = i.sync_info
        if si is None:
            continue
        for u in si.on_update:
            if u.update_mode == "sem-inc" and u.update_value == 1:
                upd_insts.setdefault(u.ant_name, []).append(i)

    # all awaited values per sem
    waited = {}
    for i in all_ins:
        si = i.sync_info
        if si is None:
            continue
        for w in si.on_wait:
            assert w.wait_reg is None
            if w.wait_mode == "sem-ge-imm":
                waited.setdefault(w.ant_name, set()).add(w.wait_value)

    for sem, insts in upd_insts.items():
        if not sem.startswith("PE_"):
            continue
        need = waited.get(sem, set())
        keep = [False] * (len(insts) + 1)  # 1-indexed ticks
        for v in need:
            assert 1 <= v <= len(insts), (sem, v, len(insts))
            keep[v] = True
        # rank[v] = kept count among ticks 1..v
        rank = [0] * (len(insts) + 1)
        acc = 0
        for v in range(1, len(insts) + 1):
            if keep[v]:
                acc += 1
            rank[v] = acc
        import concourse.mybir as _mybir
        stripped = 0
        for tick, inst in enumerate(insts, start=1):
            if keep[tick] or type(inst).__name__ != "InstMatmult":
                continue
            si = inst.sync_info
            new_upd = [u for u in si.on_update if u.ant_name != sem]
            inst.sync_info = _mybir.SyncInfo(
                on_wait=list(si.on_wait), on_update=new_upd
            )
            stripped += 1
        if stripped:
            # renumber all waits on this sem (stripped ticks were never
            # awaited, so every awaited v maps to its kept-rank exactly)
            for i in all_ins:
                si = i.sync_info
                if si is None or not si.on_wait:
                    continue
                changed = False
                wl = list(si.on_wait)
                for w in wl:
                    if w.wait_mode == "sem-ge-imm" and w.ant_name == sem:
                        nv = rank[w.wait_value]
                        if nv != w.wait_value:
                            w.wait_value = nv
                            changed = True
                if changed:
                    i.sync_info = _mybir.SyncInfo(
                        on_wait=wl, on_update=list(si.on_update)
                    )


def build_bass(t_steps=T, n_layers=L, repeat=1, elem_mode="full", whh_fp8=True,
               thin_sems=True, IDMM=False):
    """Build the SPMD per-core program. Returns nc.

    repeat > 1 re-runs the whole network that many times (for timing by
    differencing out the fixed dispatch overhead)."""
    nblk = t_steps // 32
    tok = t_steps * BC
    WHH_DT = mybir.dt.float8e4 if whh_fp8 else BF16

    nc = bacc.Bacc(None)

    # ---- external I/O ----
    x_in = nc.dram_tensor("x", [1, P, tok], BF16, kind="ExternalInput")
    ident_in = nc.dram_tensor("ident", [P, P], BF16, kind="ExternalInput")
    bsel_in = nc.dram_tensor("bsel", [32, KH * BC], BF16, kind="ExternalInput")
    wih0_in = nc.dram_tensor("wih0", [2, I, G3], BF16, kind="ExternalInput")
    whh_in = nc.dram_tensor("whh", [n_layers, 2, H, G3], WHH_DT, kind="ExternalInput")
    gxb_in = nc.dram_tensor("gxb", [n_layers, 2, P, M3], F32, kind="ExternalInput")
    bhnb_in = nc.dram_tensor("bhnb", [n_layers, 2, 32, P], BF16, kind="ExternalInput")
    out_d = nc.dram_tensor("out", [1, tok], F32, kind="ExternalOutput")
    if n_layers > 1:
        wih_in = nc.dram_tensor(
            "wih", [n_layers - 1, 2, 2 * H, G3], BF16, kind="ExternalInput"
        )
    fcw_in = nc.dram_tensor("fcw", [2 * H, 1], BF16, kind="ExternalInput")
    fcb_in = nc.dram_tensor("fcb", [1, 1], F32, kind="ExternalInput")

    # ---- internal DRAM activation ping-pong [k-chunk, 128, tok] ----
    act_a = nc.dram_tensor("act_a", [2 * KH, P, tok], BF16)
    act_b = nc.dram_tensor("act_b", [2 * KH, P, tok], BF16)

    with ChunkedDrainTC(nc) as tc:
        with (
            tc.tile_pool(name="wpool", bufs=1) as wpool,        # weights
            tc.tile_pool(name="state", bufs=1) as state,        # per-layer state
            tc.tile_pool(name="stage", bufs=3) as stage_pool,   # act staging
            tc.tile_pool(name="tmp", bufs=4) as tmp_pool,       # elementwise temps
            tc.tile_pool(name="ghps", bufs=1, space="PSUM") as ghps_pool,
            tc.tile_pool(name="gxps", bufs=3, space="PSUM") as gxps_pool,
        ):
            ident_sb = wpool.tile([P, P], BF16, tag="ident")
            nc.sync.dma_start(ident_sb[:], ident_in[:])
            for layer in [ly for _ in range(repeat) for ly in range(n_layers)]:
                ki = 1 if layer == 0 else 2 * KH  # input k-chunks
                act_in = x_in if layer == 0 else (act_a if layer % 2 == 1 else act_b)
                act_out = act_a if layer % 2 == 0 else act_b

                # ---- load weights/biases for both dirs ----
                whh_sb, wih_sb, gxb_sb, bhn_sb = [], [], [], []
                for d in range(2):
                    w = wpool.tile([P, KH, G3], WHH_DT, tag=f"whh{d}")
                    nc.sync.dma_start(
                        w[:], whh_in[layer, d].rearrange("(ko p) m -> p ko m", p=P)
                    )
                    whh_sb.append(w)
                    wi = wpool.tile([P, ki, G3], BF16, tag=f"wih{d}")
                    src = (
                        wih0_in[d]
                        if layer == 0
                        else wih_in[layer - 1, d]
                    ).rearrange("(ko p) m -> p ko m", p=P)
                    nc.sync.dma_start(wi[:], src)
                    wih_sb.append(wi)
                    gb = wpool.tile([P, M3], F32, tag=f"gxb{d}")
                    nc.sync.dma_start(gb[:], gxb_in[layer, d])
                    gxb_sb.append(gb)
                    bh = wpool.tile([32, P], BF16, tag=f"bhnb{d}")
                    nc.sync.dma_start(bh[:], bhnb_in[layer, d])
                    bhn_sb.append(bh)
                # block-diagonal indicator: ones4[p, c] = 1 iff c//BC == p, so
                # ONE rank-KH matmul adds all KH n-chunk biases:
                # out[j, c] = sum_p bhn[p, j] * ones4[p, c] = bhn[c//BC, j]
                ones4 = wpool.tile([32, KH * BC], BF16, tag="ones4")
                nc.sync.dma_start(ones4[:], bsel_in[:])

                # ---- per-layer state ----
                h_hist = []     # bf16 hidden history ring [128, KH, 2, 32*BC]
                gx_ring = []    # bf16 input-gate ring  [128, M3, 2, 32*BC]
                for d in range(2):
                    hh = state.tile([P, KH, 2, 32 * BC], BF16, tag=f"hh{d}")
                    nc.vector.memset(hh[:], 0.0)
                    h_hist.append(hh)
                    gxr = state.tile([P, M3, 2, 32 * BC], BF16, tag=f"gx{d}", name=f"gx{d}")
                    gx_ring.append(gxr)

                stage_sb = [None, None]

                def emit_gx_stage(d, tb):
                    """DMA the act tokens of t-block tb into SBUF staging."""
                    st = stage_pool.tile([P, ki, 32 * BC], BF16, tag="stage", name="st")
                    nc.sync.dma_start(
                        st[:],
                        act_in[0:ki, :, tb * 32 * BC : (tb + 1) * 32 * BC].rearrange(
                            "k p c -> p k c"
                        ),
                    )
                    stage_sb[d] = st

                def emit_gx_group(d, tb, m):
                    """Input-side gate matmuls for one m-chunk of t-block tb."""
                    par = tb % 2
                    st = stage_sb[d]
                    ps = gxps_pool.tile([P, 32 * BC], F32, tag="gxps", name="gxps")
                    for k in range(ki):
                        nc.tensor.matmul(
                            ps[:],
                            wih_sb[d][:, k, m * P : (m + 1) * P],
                            st[:, k, :],
                            start=(k == 0),
                            stop=(k == ki - 1),
                        )
                    # copy psum -> ring with per-feature bias, f32 -> bf16
                    nc.scalar.activation(
                        gx_ring[d][:, m, par, :],
                        ps[:],
                        AF.Identity,
                        bias=gxb_sb[d][:, m : m + 1],
                    )

                def emit_flush(d, tb):
                    """Store finished hidden states of t-block tb to DRAM act."""
                    par = tb % 2
                    nc.sync.dma_start(
                        act_out[
                            d * KH : (d + 1) * KH, :, tb * 32 * BC : (tb + 1) * 32 * BC
                        ].rearrange("k p c -> p k c"),
                        h_hist[d][:, :, par, :],
                    )

                def emit_step(d, t, ghrz, ghn):
                    """Recurrent matmuls for dir d. n-gate chunks go first
                    (own PSUM bank, ready early for the n path); rz chunks
                    next (own bank), then one identity matmul folds the
                    precomputed gx_rz into the rz bank so sigmoid can read
                    PSUM directly."""
                    td = t if d == 0 else (t_steps - 1 - t)  # token this step computes
                    prev = td - 1 if d == 0 else td + 1      # token holding h_{prev}
                    slp, pap = prev % 32, (prev // 32) % 2
                    sl, pa = td % 32, (td // 32) % 2

                    rhs = h_hist[d][:, :, pap, slp * BC : (slp + 1) * BC]
                    # one rank-KH matmul seeds ALL KH n-chunks with b_hn
                    # (start=True: first write to the bank this step), then
                    # every weight matmul accumulates onto it -- one clean
                    # accumulation group for the whole n side
                    nc.tensor.matmul(
                        ghn[:, 0 : KH * BC],
                        bhn_sb[d][:, :],
                        ones4[:],
                        start=True,
                        stop=False,
                        skip_group_check=True,
                    )
                    for m in range(8, M3):  # n chunks accumulate onto the bias
                        sl_ = slice((m - 8) * BC, (m - 7) * BC)
                        for k in range(KH):
                            nc.tensor.matmul(
                                ghn[:, sl_],
                                whh_sb[d][:, k, m * P : (m + 1) * P],
                                rhs[:, k, :],
                                start=False,
                                stop=(m == M3 - 1 and k == KH - 1),
                                skip_group_check=True,
                            )
                    for m in range(8):  # r,z chunks
                        sl_ = slice(m * BC, (m + 1) * BC)
                        for k in range(KH):
                            nc.tensor.matmul(
                                ghrz[:, sl_],
                                whh_sb[d][:, k, m * P : (m + 1) * P],
                                rhs[:, k, :],
                                start=(k == 0),
                                stop=(k == KH - 1),
                            )
                    if not IDMM:
                        return
                    nc.tensor.matmul(
                        ghrz[:, 0 : 8 * BC],
                        ident_sb[:],
                        gx_ring[d][:, 0:8, pa, sl * BC : (sl + 1) * BC],
                        start=False,
                        stop=True,
                        skip_group_check=True,
                    )

                def emit_elem_pair(t, grz, gn):
                    """Gate math for BOTH directions, phase-interleaved.

                    DVE and ACT are strict-FIFO: emitting dir f's whole chain
                    before dir b's parks b's ready ops behind f's unready
                    ones, serializing the two chains (measured C ~ P + 2E).
                    Interleaving by phase lets both chains run in parallel:
                      DVE: trz_f trz_b | tn_f tn_b tn2_f tn2_b | tail_f tail_b
                      ACT: sr_f sr_b sz_f sz_b | tanh_f tanh_b
                    h = n + z*(h_prev - n): rounding scales with |h_prev - n|,
                    not |h| -- no multiplicative drift on the carried state."""
                    rzv, nv, h_new, gx_n, gx_rz, h_prev = [], [], [], [], [], []
                    for d in range(2):
                        rzv.append(
                            grz[d][:, 0 : 8 * BC].rearrange("p (m b) -> p m b", b=BC)
                        )
                        nv.append(
                            gn[d][:, 0 : KH * BC].rearrange("p (m b) -> p m b", b=BC)
                        )
                        td = t if d == 0 else (t_steps - 1 - t)
                        sl, pa = td % 32, (td // 32) % 2
                        prev = td - 1 if d == 0 else td + 1
                        slp, pap = prev % 32, (prev // 32) % 2
                        h_new.append(h_hist[d][:, :, pa, sl * BC : (sl + 1) * BC])
                        gx_n.append(gx_ring[d][:, 8:12, pa, sl * BC : (sl + 1) * BC])
                        gx_rz.append(gx_ring[d][:, 0:8, pa, sl * BC : (sl + 1) * BC])
                        h_prev.append(
                            h_hist[d][:, :, pap, slp * BC : (slp + 1) * BC]
                        )
                    if elem_mode == "dummy":
                        for d in range(2):
                            nc.vector.tensor_copy(h_new[d], nv[d][:, 0:KH, :])
                        return

                    def tiles(tag):
                        return [
                            tmp_pool.tile([P, KH, BC], BF16, tag=f"{tag}{d}", name=tag)
                            for d in range(2)
                        ]

                    trz = [
                        tmp_pool.tile([P, 8, BC], BF16, tag=f"trz{d}", name="trz")
                        for d in range(2)
                    ]
                    rzin = []
                    for d in range(2):
                        if IDMM:
                            rzin.append(rzv[d])
                        else:
                            nc.vector.tensor_add(trz[d][:], rzv[d][:], gx_rz[d])
                            rzin.append(trz[d][:])
                    r_t, z_t = tiles("r"), tiles("z")
                    for d in range(2):
                        nc.scalar.activation(r_t[d][:], rzin[d][:, 0:KH, :], AF.Sigmoid)
                    for d in range(2):
                        nc.scalar.activation(
                            z_t[d][:], rzin[d][:, KH : 2 * KH, :], AF.Sigmoid
                        )
                    tn, tn2, nt = tiles("tn"), tiles("tn2"), tiles("nt")
                    for d in range(2):
                        nc.vector.tensor_mul(tn[d][:], nv[d][:], r_t[d][:])
                    for d in range(2):
                        nc.vector.tensor_add(tn2[d][:], tn[d][:], gx_n[d])
                    for d in range(2):
                        nc.scalar.activation(nt[d][:], tn2[d][:], AF.Tanh)
                    dt_, dt2 = tiles("dt"), tiles("dt2")
                    for d in range(2):
                        nc.vector.tensor_sub(dt_[d][:], h_prev[d], nt[d][:])
                        nc.vector.tensor_mul(dt2[d][:], dt_[d][:], z_t[d][:])
                        nc.vector.tensor_add(h_new[d], nt[d][:], dt2[d][:])

                # gx schedule: 24 m-groups (both dirs) spread over a block's 32
                # steps; group g of the NEXT block is emitted during step
                # ts where cumulative quota passes g.
                def gx_groups_due(ts):
                    lo = (ts * 24) // 32
                    hi = ((ts + 1) * 24) // 32
                    return range(lo, hi)

                # ---- prologue: stage + gx for the first consumed blocks ----
                emit_gx_stage(0, 0)
                emit_gx_stage(1, nblk - 1)
                for m in range(M3):
                    emit_gx_group(0, 0, m)
                    emit_gx_group(1, nblk - 1, m)
                for tb in range(nblk):
                    if tb < nblk - 1:
                        emit_gx_stage(0, tb + 1)
                        emit_gx_stage(1, nblk - 2 - tb)
                    for ts in range(32):
                        t = tb * 32 + ts
                        # separate PSUM banks per (dir, gate-group) so each
                        # consumer only waits for its own producer matmuls
                        grz0 = ghps_pool.tile([P, 512], F32, tag="grz0", name="grz0")
                        gn0 = ghps_pool.tile([P, 512], F32, tag="gn0", name="gn0")
                        grz1 = ghps_pool.tile([P, 512], F32, tag="grz1", name="grz1")
                        gn1 = ghps_pool.tile([P, 512], F32, tag="gn1", name="gn1")
                        emit_step(0, t, grz0, gn0)
                        emit_step(1, t, grz1, gn1)
                        if tb < nblk - 1:
                            for g in gx_groups_due(ts):
                                d, m = g % 2, g // 2
                                emit_gx_group(d, tb + 1 if d == 0 else nblk - 2 - tb, m)
                        emit_elem_pair(t, [grz0, grz1], [gn0, gn1])
                    emit_flush(0, tb)
                    emit_flush(1, nblk - 1 - tb)

            # ---- final FC + sigmoid over the last layer's output ----
            act_fin = act_a if (n_layers - 1) % 2 == 0 else act_b
            fcw_sb = wpool.tile([P, 2 * KH, 1], BF16, tag="fcw")
            nc.sync.dma_start(fcw_sb[:], fcw_in.rearrange("(ko p) n -> p ko n", p=P))
            fcb_sb = wpool.tile([1, 1], F32, tag="fcb")
            nc.sync.dma_start(fcb_sb[:], fcb_in[:])
            out_sb = state.tile([1, tok], F32, tag="osb")
            for blk in range(nblk):
                st = stage_pool.tile([P, 2 * KH, 32 * BC], BF16, tag="stage")
                nc.sync.dma_start(
                    st[:],
                    act_fin[:, :, blk * 32 * BC : (blk + 1) * 32 * BC].rearrange(
                        "k p c -> p k c"
                    ),
                )
                ps = gxps_pool.tile([1, 32 * BC], F32, tag="gxps")
                for k in range(2 * KH):
                    nc.tensor.matmul(
                        ps[:],
                        fcw_sb[:, k, :],
                        st[:, k, :],
                        start=(k == 0),
                        stop=(k == 2 * KH - 1),
                    )
                nc.scalar.activation(
                    out_sb[:, blk * 32 * BC : (blk + 1) * 32 * BC],
                    ps[:],
                    AF.Sigmoid,
                    bias=fcb_sb[:, 0:1],
                )
            nc.sync.dma_start(out_d[:], out_sb[:])

    nc.finalize()
    if thin_sems:
        thin_pe_sems(nc)
    return nc


def prep_inputs(input_seq, W_ih0, W_hh0, b_ih0, b_hh0, W_ih, W_hh, b_ih, b_hh,
                fc_w, fc_b, t_steps=T, n_layers=L, whh_fp8=True):
    """Host-side prep: transposes, bias folding, bf16 casts. Returns in_maps."""
    bf = ml_dtypes.bfloat16
    whh_dt = ml_dtypes.float8_e4m3fn if whh_fp8 else bf
    tok = t_steps * BC

    wih0 = np.ascontiguousarray(np.transpose(np.asarray(W_ih0), (0, 2, 1))).astype(bf)
    whh_all = np.concatenate(
        [np.asarray(W_hh0)[None], np.asarray(W_hh)], axis=0
    )[:n_layers]
    whh = np.ascontiguousarray(np.transpose(whh_all, (0, 1, 3, 2))).astype(whh_dt)
    bih_all = np.concatenate([np.asarray(b_ih0)[None], np.asarray(b_ih)], axis=0)[:n_layers]
    bhh_all = np.concatenate([np.asarray(b_hh0)[None], np.asarray(b_hh)], axis=0)[:n_layers]

    # gx bias: b_ih everywhere + b_hh on the r,z gates only (b_hn rides separately)
    gxb = bih_all.copy()
    gxb[:, :, : 2 * H] += bhh_all[:, :, : 2 * H]
    gxb = np.ascontiguousarray(
        np.transpose(gxb.reshape(n_layers, 2, M3, P), (0, 1, 3, 2))
    ).astype(np.float32)
    # K rows padded to 32: tile_size rounds K up to 32 and the PE streams
    # all 32 rows, so rows KH..31 must be real zeros
    bhnb = np.zeros((n_layers, 2, 32, 128), np.float32)
    bhnb[:, :, : H // 128] = bhh_all[:, :, 2 * H :].reshape(
        n_layers, 2, H // 128, 128
    )
    bhnb = bhnb.astype(bf)

    base = {
        "wih0": wih0,
        "whh": whh,
        "gxb": gxb,
        "bhnb": bhnb,
        "ident": np.eye(P, dtype=bf),
        "bsel": np.concatenate([
            np.kron(np.eye(H // 128, dtype=np.float32),
                    np.ones((1, BC), np.float32)),
            np.zeros((32 - H // 128, (H // 128) * BC), np.float32),
        ]).astype(bf),
        "fcw": np.ascontiguousarray(np.asarray(fc_w).T).astype(bf),
        "fcb": np.asarray(fc_b, dtype=np.float32).reshape(1, 1),
    }
    if n_layers > 1:
        base["wih"] = np.ascontiguousarray(
            np.transpose(np.asarray(W_ih), (0, 1, 3, 2))
        )[: n_layers - 1].astype(bf)

    x = np.asarray(input_seq)[:t_steps]
    in_maps = []
    for c in range(NCORES):
        xc = x[:, c * BC : (c + 1) * BC, :].reshape(tok, I).T  # [128, tok]
        m = dict(base)
        m["x"] = np.ascontiguousarray(xc)[None].astype(bf)
        in_maps.append(m)
    return in_maps


def assemble_output(results, t_steps=T):
    """results: list of per-core dicts with 'out' [1, tok] -> [T, B, 1] f32."""
    outs = []
    for c in range(NCORES):
        o = np.asarray(results[c]["out"]).reshape(t_steps, BC)
        outs.append(o)
    return np.stack(outs, axis=1).reshape(t_steps, B)[:, :, None].astype(np.float32)


def kernel(**inputs):
    nc = build_bass()
    in_maps = prep_inputs(**inputs)
    res = run_bass_kernel_spmd(nc, in_maps, list(range(NCORES)))
    return assemble_output(res.results)

